# revision 7
# baseline (speedup 1.0000x reference)
"""Supervised contrastive loss (nn_Batch_CL) on 8 Trainium2 NeuronCores.

Math (per the reference):
  x = l2_normalize(feature_embeds)            # [N, D]
  logits = (x @ x.T) / tau                    # tau = 0.1
  Z_i    = sum_{j != i} exp(logits[i, j])
  S_i    = sum_{j != i, l_j == l_i} logits[i, j]
  P_i    = |{j != i : l_j == l_i}|
  per_row_i = S_i / P_i - log Z_i   (if P_i > 0 else 0)
  loss = -sum(per_row) / n_valid

Distribution: rows sharded 8 ways (1024 rows/core). Each core receives the
full feature matrix with ITS OWN rows permuted to the front, so the diagonal
of its logits block lands at a statically-known position (cols m*128..m*128+127
of column-group 0 for row-chunk m) — no core-id branching is needed; the SPMD
program is identical and only the input data differs.

Per-core kernel strategy:
  - exp+row-sum fused in one ACT instruction per [128, 1024] PSUM block via
    activation(Exp, scale=10, accum_out=...): the Z reduction is free.
  - positive-pair sums via class aggregation: Msum = x_hat^T @ onehot(labels)
    accumulated on PE, then F = x_hat_block @ Msum gives per-(row, class)
    sums; selecting the row's own class with a one-hot mask + accum_out
    yields S_i without any NxN mask work.
  - exact diagonal terms extracted from the PSUM logits blocks with an
    identity-mask scalar_tensor_tensor + accum_out, so Z_i excludes e^{l_ii}
    bit-exactly and S_i excludes l_ii.
  - l2 normalization: rsqrt(s) = Exp(-0.5 * Log(s)) on ACT — stays in the
    natural_log_exp table set used by the main exp (no table thrash).
  - x^T (contraction layout) built with bf16 DMA-xbar transposes.

Outputs per core: [sum of valid per_row over its 1024 rows, its n_valid].
Host epilogue: loss = -sum(parts) / sum(n_valid).
"""

import numpy as np

N = 8192
D = 128
N_CORES = 8
ROWS_PER_CORE = N // N_CORES          # 1024
NCHUNK = N // 128                     # 64 chunks of 128 rows
NGROUP = 8                            # column groups
GW = N // NGROUP                      # 1024 cols per group
CHUNKS_PER_GROUP = GW // 128          # 8
NOWN = ROWS_PER_CORE // 128           # 8 own row-chunks
NCLS = 33
INV_TAU = 10.0

_NC = None


def _build_nc(split_waits=True):
    import concourse.bass as bass
    import concourse.mybir as mybir
    from concourse import tile
    from contextlib import ExitStack
    import tile_patch

    tile_patch.install()

    f32 = mybir.dt.float32
    bf16 = mybir.dt.bfloat16
    Alu = mybir.AluOpType
    Act = mybir.ActivationFunctionType

    nc = bass.Bass()
    x_dram = nc.dram_tensor("xperm", [N, D], f32, kind="ExternalInput")
    lab_dram = nc.dram_tensor("labels_pc", [128, NCHUNK], f32, kind="ExternalInput")
    iota_dram = nc.dram_tensor("iota33", [128, NCLS], f32, kind="ExternalInput")
    eye_dram = nc.dram_tensor("identity", [128, 128], f32, kind="ExternalInput")
    out_dram = nc.dram_tensor("out", [2], f32, kind="ExternalOutput")
    dbg_zpart = nc.dram_tensor("dbg_zpart", [128, NGROUP * NOWN], f32, kind="ExternalOutput")
    dbg_rawdiag = nc.dram_tensor("dbg_rawdiag", [128, NOWN], f32, kind="ExternalOutput")
    dbg_pown = nc.dram_tensor("dbg_pown", [128, NOWN], f32, kind="ExternalOutput")
    dbg_sfull = nc.dram_tensor("dbg_sfull", [128, NOWN], f32, kind="ExternalOutput")
    dbg_cnt = nc.dram_tensor("dbg_cnt", [128, NCLS], f32, kind="ExternalOutput")
    dbg_xt = nc.dram_tensor("dbg_xt", [128, 256], f32, kind="ExternalOutput")
    dbg_parts = nc.dram_tensor("dbg_parts", [128, 2], f32, kind="ExternalOutput")

    with tile.TileContext(nc) as tc, ExitStack() as ctx:
        persist = ctx.enter_context(tc.tile_pool(name="persist", bufs=1))

        xT = persist.tile([128, N], bf16)                 # normalized, transposed
        O_bf = persist.tile([128, NCHUNK * NCLS], bf16)   # one-hot labels (PE operand)
        O_own = persist.tile([128, NOWN * NCLS], f32)     # one-hot, own chunks (DVE)
        cnt_bcast = persist.tile([128, NCLS], f32)        # class counts, bcast
        Zpart = persist.tile([128, NGROUP * NOWN], f32)   # exp row-sum partials
        rawdiag = persist.tile([128, NOWN], f32)          # logits diagonal (pre-exp)
        P_own = persist.tile([128, NOWN], f32)            # count[label(row)]
        S_full = persist.tile([128, NOWN], f32)           # class-sum at own label
        labels_sb = persist.tile([128, NCHUNK], f32)
        iota_sb = persist.tile([128, NCLS], f32)
        eye_sb = persist.tile([128, 128], f32)
        ones_f = persist.tile([128, 1], f32)
        ones_bf = persist.tile([128, 1], bf16)
        ones_row = persist.tile([1, 128], f32)
        cnt_row = persist.tile([1, NCLS], f32)
        Mt_sb = persist.tile([128, NCLS], bf16)           # Msum (class sums of x_hat)
        dump128 = persist.tile([128, 128], f32)           # STT writeback scratch
        dump33 = persist.tile([128, NCLS], f32)
        e_dump = persist.tile([128, GW], f32)             # ACT out scratch (unread)
        res_sb = persist.tile([1, 2], f32)

        # epilogue tiles
        Zrow = persist.tile([128, NOWN], f32)
        e_diag = persist.tile([128, NOWN], f32)
        Zexcl = persist.tile([128, NOWN], f32)
        lnZ = persist.tile([128, NOWN], f32)
        S_excl = persist.tile([128, NOWN], f32)
        P_pos = persist.tile([128, NOWN], f32)
        P_safe = persist.tile([128, NOWN], f32)
        P_inv = persist.tile([128, NOWN], f32)
        valid = persist.tile([128, NOWN], f32)
        t_sp = persist.tile([128, NOWN], f32)
        perrow = persist.tile([128, NOWN], f32)
        loss_parts = persist.tile([128, 2], f32)

        # ---------------- prologue ----------------
        nc.sync.dma_start(labels_sb[:], lab_dram[:])
        nc.sync.dma_start(iota_sb[:], iota_dram[:])
        nc.sync.dma_start(eye_sb[:], eye_dram[:])
        nc.vector.memset(ones_f[:], 1.0)
        nc.vector.memset(Zpart[:], 0.0)
        nc.vector.memset(rawdiag[:], 0.0)
        nc.vector.memset(P_own[:], 0.0)
        nc.vector.memset(S_full[:], 0.0)
        nc.vector.memset(ones_bf[:], 1.0)
        nc.vector.memset(ones_row[:], 1.0)

        for c in range(NCHUNK):
            nc.vector.tensor_scalar(
                out=O_bf[:, c * NCLS:(c + 1) * NCLS],
                in0=iota_sb[:],
                scalar1=labels_sb[:, c:c + 1],
                scalar2=None,
                op0=Alu.is_equal,
            )
        for m in range(NOWN):
            nc.vector.tensor_scalar(
                out=O_own[:, m * NCLS:(m + 1) * NCLS],
                in0=iota_sb[:],
                scalar1=labels_sb[:, m:m + 1],
                scalar2=None,
                op0=Alu.is_equal,
            )

        with tc.tile_pool(name="pro_ps", bufs=1, space="PSUM") as pro_ps:
            cnt_ps = pro_ps.tile([1, NCLS], f32, tag="cnt")
            for c in range(NCHUNK):
                nc.tensor.matmul(
                    cnt_ps[:], ones_bf[:], O_bf[:, c * NCLS:(c + 1) * NCLS],
                    start=(c == 0), stop=(c == NCHUNK - 1),
                )
            nc.vector.tensor_copy(cnt_row[:], cnt_ps[:])
            cntb_ps = pro_ps.tile([128, NCLS], f32, tag="cntb")
            nc.tensor.matmul(cntb_ps[:], ones_row[:], cnt_row[:], start=True, stop=True)
            nc.vector.tensor_copy(cnt_bcast[:], cntb_ps[:])

        for m in range(NOWN):
            nc.vector.scalar_tensor_tensor(
                out=dump33[:],
                in0=O_own[:, m * NCLS:(m + 1) * NCLS],
                scalar=1.0,
                in1=cnt_bcast[:],
                op0=Alu.mult,
                op1=Alu.mult,
                accum_out=P_own[:, m:m + 1],
            )

        # ---------------- main: build + compute, group by group ----------------
        with (
            tc.tile_pool(name="main_ps", bufs=3, space="PSUM") as main_ps,
            tc.tile_pool(name="msum_ps", bufs=1, space="PSUM") as msum_pool,
            tc.tile_pool(name="build", bufs=2) as build_pool,
        ):
            msum_ps = msum_pool.tile([128, NCLS], f32)
            for g in range(NGROUP):
                # --- build group g of xT (normalize + cast + transpose) ---
                xs = build_pool.tile([128, GW], f32, tag="xs")
                nc.sync.dma_start(
                    xs[:].rearrange("p (c d) -> p c d", d=128),
                    x_dram[g * GW:(g + 1) * GW, :].rearrange("(c p) d -> p c d", p=128),
                )
                sq = build_pool.tile([128, GW], f32, tag="sq")
                nc.vector.tensor_mul(sq[:], xs[:], xs[:])
                ssq = build_pool.tile([128, CHUNKS_PER_GROUP], f32, tag="ssq")
                nc.vector.reduce_sum(
                    ssq[:], sq[:].rearrange("p (c d) -> p c d", d=128),
                    axis=mybir.AxisListType.X,
                )
                lns = build_pool.tile([128, CHUNKS_PER_GROUP], f32, tag="lns")
                nc.scalar.activation(lns[:], ssq[:], Act.Ln)
                rinv = build_pool.tile([128, CHUNKS_PER_GROUP], f32, tag="rinv")
                nc.scalar.activation(rinv[:], lns[:], Act.Exp, scale=-0.5)
                xh = build_pool.tile([128, GW], bf16, tag="xh")
                for i in range(CHUNKS_PER_GROUP):
                    c = g * CHUNKS_PER_GROUP + i
                    sl = slice(i * 128, (i + 1) * 128)
                    nc.vector.tensor_scalar_mul(xh[:, sl], xs[:, sl], rinv[:, i:i + 1])
                    nc.sync.dma_start_transpose(
                        xT[:, c * 128:(c + 1) * 128], xh[:, sl]
                    )
                    nc.tensor.matmul(
                        msum_ps[:], xh[:, sl], O_bf[:, c * NCLS:(c + 1) * NCLS],
                        start=(c == 0), stop=(c == NCHUNK - 1),
                    )

                # --- logits + exp + rowsum for all 8 own row-chunks ---
                for m in range(NOWN):
                    ps = main_ps.tile([128, GW], f32, tag="e")
                    lhsT = xT[:, m * 128:(m + 1) * 128]
                    for k in range(GW // 512):
                        nc.tensor.matmul(
                            ps[:, k * 512:(k + 1) * 512],
                            lhsT,
                            xT[:, g * GW + k * 512: g * GW + (k + 1) * 512],
                            start=True, stop=True,
                        )
                    if g == 0:
                        nc.vector.scalar_tensor_tensor(
                            out=dump128[:],
                            in0=ps[:, m * 128:(m + 1) * 128],
                            scalar=1.0,
                            in1=eye_sb[:],
                            op0=Alu.mult,
                            op1=Alu.mult,
                            accum_out=rawdiag[:, m:m + 1],
                        )
                    nc.scalar.activation(
                        e_dump[:], ps[:], Act.Exp, scale=INV_TAU,
                        accum_out=Zpart[:, g * NOWN + m: g * NOWN + m + 1],
                    )

            nc.vector.tensor_copy(Mt_sb[:], msum_ps[:])

        # ---------------- epilogue ----------------
        with tc.tile_pool(name="epi_ps", bufs=1, space="PSUM") as epi_ps:
            F_ps = epi_ps.tile([128, NOWN * NCLS], f32, tag="F")
            for m in range(NOWN):
                nc.tensor.matmul(
                    F_ps[:, m * NCLS:(m + 1) * NCLS],
                    xT[:, m * 128:(m + 1) * 128],
                    Mt_sb[:],
                    start=True, stop=True,
                )
            for m in range(NOWN):
                nc.vector.scalar_tensor_tensor(
                    out=dump33[:],
                    in0=F_ps[:, m * NCLS:(m + 1) * NCLS],
                    scalar=1.0,
                    in1=O_own[:, m * NCLS:(m + 1) * NCLS],
                    op0=Alu.mult,
                    op1=Alu.mult,
                    accum_out=S_full[:, m:m + 1],
                )

            # Z excluding the diagonal, matching the in-matrix rounding exactly
            nc.vector.reduce_sum(
                Zrow[:], Zpart[:].rearrange("p (g m) -> p m g", m=NOWN),
                axis=mybir.AxisListType.X,
            )
            nc.scalar.activation(e_diag[:], rawdiag[:], Act.Exp, scale=INV_TAU)
            nc.vector.tensor_sub(Zexcl[:], Zrow[:], e_diag[:])
            nc.scalar.activation(lnZ[:], Zexcl[:], Act.Ln)

            # S excluding diagonal, scaled by 1/tau; positive count P
            nc.vector.tensor_sub(S_excl[:], S_full[:], rawdiag[:])
            nc.vector.tensor_scalar_add(P_pos[:], P_own[:], -1.0)
            nc.vector.tensor_scalar_max(P_safe[:], P_pos[:], 1.0)
            nc.vector.reciprocal(P_inv[:], P_safe[:])
            nc.vector.tensor_scalar(
                out=valid[:], in0=P_pos[:], scalar1=0.5, scalar2=None, op0=Alu.is_ge
            )
            # per_row = valid * (S*10*Pinv - lnZ)
            nc.vector.scalar_tensor_tensor(
                out=t_sp[:], in0=S_excl[:], scalar=INV_TAU, in1=P_inv[:],
                op0=Alu.mult, op1=Alu.mult,
            )
            nc.vector.tensor_sub(perrow[:], t_sp[:], lnZ[:])
            nc.vector.tensor_mul(perrow[:], perrow[:], valid[:])

            nc.vector.reduce_sum(
                loss_parts[:, 0:1], perrow[:],
                axis=mybir.AxisListType.X,
            )
            nc.vector.reduce_sum(
                loss_parts[:, 1:2], valid[:],
                axis=mybir.AxisListType.X,
            )
            sum_ps = epi_ps.tile([1, 2], f32, tag="sum")
            nc.tensor.matmul(sum_ps[:], ones_f[:], loss_parts[:], start=True, stop=True)
            nc.vector.tensor_copy(res_sb[:], sum_ps[:])
            nc.sync.dma_start(out_dram[:].rearrange("(a b) -> a b", a=1), res_sb[:])
            nc.sync.dma_start(dbg_zpart[:], Zpart[:])
            nc.sync.dma_start(dbg_rawdiag[:], rawdiag[:])
            nc.sync.dma_start(dbg_pown[:], P_own[:])
            nc.sync.dma_start(dbg_sfull[:], S_full[:])
            nc.sync.dma_start(dbg_cnt[:], cnt_bcast[:])
            nc.sync.dma_start(dbg_parts[:], loss_parts[:])
            xt_dbg = persist.tile([128, 256], f32)
            nc.vector.tensor_copy(xt_dbg[:], xT[:, 0:256])
            nc.sync.dma_start(dbg_xt[:], xt_dbg[:])

    if split_waits:
        tile_patch.split_multiwait(nc)
    return nc


def _get_nc(split_waits=True):
    global _NC
    if _NC is None:
        _NC = _build_nc(split_waits)
    return _NC


def _make_in_maps(x, lab):
    iota = np.ascontiguousarray(
        np.tile(np.arange(NCLS, dtype=np.float32), (128, 1))
    )
    eye = np.eye(128, dtype=np.float32)
    in_maps = []
    for c in range(N_CORES):
        lo, hi = c * ROWS_PER_CORE, (c + 1) * ROWS_PER_CORE
        perm = np.concatenate(
            [np.arange(lo, hi), np.arange(0, lo), np.arange(hi, N)]
        )
        xp = np.ascontiguousarray(x[perm])
        lp = np.ascontiguousarray(
            lab[perm].astype(np.float32).reshape(NCHUNK, 128).T
        )
        in_maps.append(
            {"xperm": xp, "labels_pc": lp, "iota33": iota, "identity": eye}
        )
    return in_maps


def _combine(results):
    parts = np.stack([np.asarray(results[c]["out"]) for c in range(N_CORES)])
    loss = -parts[:, 0].sum() / parts[:, 1].sum()
    return np.array(loss, dtype=np.float32)


def kernel(feature_embeds, label_ids):
    from concourse.bass_utils import run_bass_kernel_spmd

    x = np.asarray(feature_embeds, dtype=np.float32)
    lab = np.asarray(label_ids)
    nc = _get_nc()
    res = run_bass_kernel_spmd(nc, _make_in_maps(x, lab), list(range(N_CORES)))
    return _combine(res.results)


def kernel_profiled(feature_embeds, label_ids):
    """Same as kernel(), but with NTFF tracing; returns (loss, exec_time_ns)."""
    import profile_hook
    print("ntff hook installed:", profile_hook.install())
    from concourse.bass_utils import run_bass_kernel_spmd

    x = np.asarray(feature_embeds, dtype=np.float32)
    lab = np.asarray(label_ids)
    nc = _get_nc()
    res = run_bass_kernel_spmd(
        nc, _make_in_maps(x, lab), list(range(N_CORES)), trace=True
    )
    return _combine(res.results), res.exec_time_ns


# revision 8
# speedup vs baseline: 1.3620x; 1.3620x over previous
"""Supervised contrastive loss (nn_Batch_CL) on 8 Trainium2 NeuronCores.

Math (per the reference):
  x = l2_normalize(feature_embeds)            # [N, D]
  logits = (x @ x.T) / tau                    # tau = 0.1
  Z_i    = sum_{j != i} exp(logits[i, j])
  S_i    = sum_{j != i, l_j == l_i} logits[i, j]
  P_i    = |{j != i : l_j == l_i}|
  per_row_i = S_i / P_i - log Z_i   (if P_i > 0 else 0)
  loss = -sum(per_row) / n_valid

Distribution: rows sharded 8 ways (1024 rows/core). Each core receives the
full feature matrix with ITS OWN rows permuted to the front, so the diagonal
of its logits block lands at a statically-known position (cols m*128..m*128+127
of column-group 0 for row-chunk m) — no core-id branching is needed; the SPMD
program is identical and only the input data differs.

Per-core kernel strategy:
  - exp+row-sum fused in one ACT instruction per [128, 1024] PSUM block via
    activation(Exp, scale=10, accum_out=...): the Z reduction is free.
  - positive-pair sums via class aggregation: Msum = x_hat^T @ onehot(labels)
    accumulated on PE, then F = x_hat_block @ Msum gives per-(row, class)
    sums; selecting the row's own class with a one-hot mask + accum_out
    yields S_i without any NxN mask work.
  - exact diagonal terms extracted from the PSUM logits blocks with an
    identity-mask scalar_tensor_tensor + accum_out, so Z_i excludes e^{l_ii}
    bit-exactly and S_i excludes l_ii.
  - l2 normalization: rsqrt(s) = Exp(-0.5 * Log(s)) on ACT — stays in the
    natural_log_exp table set used by the main exp (no table thrash).
  - x^T (contraction layout) built with bf16 DMA-xbar transposes.

Outputs per core: [sum of valid per_row over its 1024 rows, its n_valid].
Host epilogue: loss = -sum(parts) / sum(n_valid).
"""

import numpy as np

N = 8192
D = 128
N_CORES = 8
ROWS_PER_CORE = N // N_CORES          # 1024
NCHUNK = N // 128                     # 64 chunks of 128 rows
NGROUP = 8                            # column groups
GW = N // NGROUP                      # 1024 cols per group
CHUNKS_PER_GROUP = GW // 128          # 8
NOWN = ROWS_PER_CORE // 128           # 8 own row-chunks
NCLS = 33
INV_TAU = 10.0

_NC = None


def _build_nc(split_waits=True):
    import concourse.bass as bass
    import concourse.mybir as mybir
    from concourse import tile
    from contextlib import ExitStack
    import tile_patch

    tile_patch.install()

    f32 = mybir.dt.float32
    bf16 = mybir.dt.bfloat16
    Alu = mybir.AluOpType
    Act = mybir.ActivationFunctionType

    nc = bass.Bass()
    x_dram = nc.dram_tensor("xperm", [N, D], f32, kind="ExternalInput")
    lab_dram = nc.dram_tensor("labels_pc", [128, NCHUNK], f32, kind="ExternalInput")
    iota_dram = nc.dram_tensor("iota33", [128, NCLS], f32, kind="ExternalInput")
    eye_dram = nc.dram_tensor("identity", [128, 128], f32, kind="ExternalInput")
    out_dram = nc.dram_tensor("out", [2], f32, kind="ExternalOutput")
    dbg_zpart = nc.dram_tensor("dbg_zpart", [128, NGROUP * NOWN], f32, kind="ExternalOutput")
    dbg_rawdiag = nc.dram_tensor("dbg_rawdiag", [128, NOWN], f32, kind="ExternalOutput")
    dbg_pown = nc.dram_tensor("dbg_pown", [128, NOWN], f32, kind="ExternalOutput")
    dbg_sfull = nc.dram_tensor("dbg_sfull", [128, NOWN], f32, kind="ExternalOutput")
    dbg_cnt = nc.dram_tensor("dbg_cnt", [128, NCLS], f32, kind="ExternalOutput")
    dbg_xt = nc.dram_tensor("dbg_xt", [128, 256], f32, kind="ExternalOutput")
    dbg_parts = nc.dram_tensor("dbg_parts", [128, 2], f32, kind="ExternalOutput")

    with tile.TileContext(nc) as tc, ExitStack() as ctx:
        persist = ctx.enter_context(tc.tile_pool(name="persist", bufs=1))

        xT = persist.tile([128, N], bf16)                 # normalized, transposed
        O_bf = persist.tile([128, NCHUNK * NCLS], bf16)   # one-hot labels (PE operand)
        O_own = persist.tile([128, NOWN * NCLS], f32)     # one-hot, own chunks (DVE)
        cnt_bcast = persist.tile([128, NCLS], f32)        # class counts, bcast
        Zpart = persist.tile([128, NGROUP * NOWN], f32)   # exp row-sum partials
        rawdiag = persist.tile([128, NOWN], f32)          # logits diagonal (pre-exp)
        P_own = persist.tile([128, NOWN], f32)            # count[label(row)]
        S_full = persist.tile([128, NOWN], f32)           # class-sum at own label
        labels_sb = persist.tile([128, NCHUNK], f32)
        iota_sb = persist.tile([128, NCLS], f32)
        eye_sb = persist.tile([128, 128], f32)
        ones_f = persist.tile([128, 1], f32)
        ones_bf = persist.tile([128, 1], bf16)
        ones_row = persist.tile([1, 128], f32)
        cnt_row = persist.tile([1, NCLS], f32)
        Mt_sb = persist.tile([128, NCLS], bf16)           # Msum (class sums of x_hat)
        dump128 = persist.tile([128, 128], f32)           # STT writeback scratch
        dump33 = persist.tile([128, NCLS], f32)
        e_dump = persist.tile([128, GW], f32)             # ACT out scratch (unread)
        res_sb = persist.tile([1, 2], f32)

        # epilogue tiles
        Zrow = persist.tile([128, NOWN], f32)
        e_diag = persist.tile([128, NOWN], f32)
        Zexcl = persist.tile([128, NOWN], f32)
        lnZ = persist.tile([128, NOWN], f32)
        S_excl = persist.tile([128, NOWN], f32)
        P_pos = persist.tile([128, NOWN], f32)
        P_safe = persist.tile([128, NOWN], f32)
        P_inv = persist.tile([128, NOWN], f32)
        valid = persist.tile([128, NOWN], f32)
        t_sp = persist.tile([128, NOWN], f32)
        perrow = persist.tile([128, NOWN], f32)
        loss_parts = persist.tile([128, 2], f32)

        # ---------------- prologue ----------------
        nc.sync.dma_start(labels_sb[:], lab_dram[:])
        nc.sync.dma_start(iota_sb[:], iota_dram[:])
        nc.sync.dma_start(eye_sb[:], eye_dram[:])
        nc.vector.memset(ones_f[:], 1.0)
        nc.vector.memset(Zpart[:], 0.0)
        nc.vector.memset(rawdiag[:], 0.0)
        nc.vector.memset(P_own[:], 0.0)
        nc.vector.memset(S_full[:], 0.0)
        nc.vector.memset(ones_bf[:], 1.0)
        nc.vector.memset(ones_row[:], 1.0)

        for c in range(NCHUNK):
            nc.vector.tensor_scalar(
                out=O_bf[:, c * NCLS:(c + 1) * NCLS],
                in0=iota_sb[:],
                scalar1=labels_sb[:, c:c + 1],
                scalar2=None,
                op0=Alu.is_equal,
            )
        for m in range(NOWN):
            nc.vector.tensor_scalar(
                out=O_own[:, m * NCLS:(m + 1) * NCLS],
                in0=iota_sb[:],
                scalar1=labels_sb[:, m:m + 1],
                scalar2=None,
                op0=Alu.is_equal,
            )

        with tc.tile_pool(name="pro_ps", bufs=1, space="PSUM") as pro_ps:
            cnt_ps = pro_ps.tile([1, NCLS], f32, tag="cnt")
            for c in range(NCHUNK):
                nc.tensor.matmul(
                    cnt_ps[:], ones_bf[:], O_bf[:, c * NCLS:(c + 1) * NCLS],
                    start=(c == 0), stop=(c == NCHUNK - 1),
                )
            nc.vector.tensor_copy(cnt_row[:], cnt_ps[:])
            cntb_ps = pro_ps.tile([128, NCLS], f32, tag="cntb")
            nc.tensor.matmul(cntb_ps[:], ones_row[:], cnt_row[:], start=True, stop=True)
            nc.vector.tensor_copy(cnt_bcast[:], cntb_ps[:])

        for m in range(NOWN):
            nc.vector.scalar_tensor_tensor(
                out=dump33[:],
                in0=O_own[:, m * NCLS:(m + 1) * NCLS],
                scalar=1.0,
                in1=cnt_bcast[:],
                op0=Alu.mult,
                op1=Alu.mult,
                accum_out=P_own[:, m:m + 1],
            )

        # ---------------- main: build + compute, group by group ----------------
        with (
            tc.tile_pool(name="main_ps", bufs=3, space="PSUM") as main_ps,
            tc.tile_pool(name="msum_ps", bufs=1, space="PSUM") as msum_pool,
            tc.tile_pool(name="build", bufs=2) as build_pool,
        ):
            msum_ps = msum_pool.tile([128, NCLS], f32)
            for g in range(NGROUP):
                # --- build group g of xT (normalize + cast + transpose) ---
                xs = build_pool.tile([128, GW], f32, tag="xs")
                nc.sync.dma_start(
                    xs[:].rearrange("p (c d) -> p c d", d=128),
                    x_dram[g * GW:(g + 1) * GW, :].rearrange("(c p) d -> p c d", p=128),
                )
                sq = build_pool.tile([128, GW], f32, tag="sq")
                nc.vector.tensor_mul(sq[:], xs[:], xs[:])
                ssq = build_pool.tile([128, CHUNKS_PER_GROUP], f32, tag="ssq")
                nc.vector.reduce_sum(
                    ssq[:], sq[:].rearrange("p (c d) -> p c d", d=128),
                    axis=mybir.AxisListType.X,
                )
                lns = build_pool.tile([128, CHUNKS_PER_GROUP], f32, tag="lns")
                nc.scalar.activation(lns[:], ssq[:], Act.Ln)
                rinv = build_pool.tile([128, CHUNKS_PER_GROUP], f32, tag="rinv")
                nc.scalar.activation(rinv[:], lns[:], Act.Exp, scale=-0.5)
                xh = build_pool.tile([128, GW], bf16, tag="xh")
                nc.vector.scalar_tensor_tensor(
                    out=xh[:].rearrange("p (c r) -> p c r", r=128),
                    in0=xs[:].rearrange("p (c r) -> p c r", r=128),
                    scalar=1.0,
                    in1=rinv[:].to_broadcast((128, CHUNKS_PER_GROUP, 128)),
                    op0=Alu.mult,
                    op1=Alu.mult,
                )
                nc.sync.dma_start_transpose(
                    xT[:, g * GW:(g + 1) * GW].rearrange("p (c r) -> p c r", r=128),
                    xh[:],
                )
                for i in range(CHUNKS_PER_GROUP):
                    c = g * CHUNKS_PER_GROUP + i
                    sl = slice(i * 128, (i + 1) * 128)
                    nc.tensor.matmul(
                        msum_ps[:], xh[:, sl], O_bf[:, c * NCLS:(c + 1) * NCLS],
                        start=(c == 0), stop=(c == NCHUNK - 1),
                    )

                # --- logits + exp + rowsum for all 8 own row-chunks ---
                for m in range(NOWN):
                    ps = main_ps.tile([128, GW], f32, tag="e")
                    lhsT = xT[:, m * 128:(m + 1) * 128]
                    for k in range(GW // 512):
                        nc.tensor.matmul(
                            ps[:, k * 512:(k + 1) * 512],
                            lhsT,
                            xT[:, g * GW + k * 512: g * GW + (k + 1) * 512],
                            start=True, stop=True,
                        )
                    if g == 0:
                        nc.vector.scalar_tensor_tensor(
                            out=dump128[:],
                            in0=ps[:, m * 128:(m + 1) * 128],
                            scalar=1.0,
                            in1=eye_sb[:],
                            op0=Alu.mult,
                            op1=Alu.mult,
                            accum_out=rawdiag[:, m:m + 1],
                        )
                    nc.scalar.activation(
                        e_dump[:], ps[:], Act.Exp, scale=INV_TAU,
                        accum_out=Zpart[:, g * NOWN + m: g * NOWN + m + 1],
                    )

            nc.vector.tensor_copy(Mt_sb[:], msum_ps[:])

        # ---------------- epilogue ----------------
        with tc.tile_pool(name="epi_ps", bufs=1, space="PSUM") as epi_ps:
            F_ps = epi_ps.tile([128, NOWN * NCLS], f32, tag="F")
            for m in range(NOWN):
                nc.tensor.matmul(
                    F_ps[:, m * NCLS:(m + 1) * NCLS],
                    xT[:, m * 128:(m + 1) * 128],
                    Mt_sb[:],
                    start=True, stop=True,
                )
            for m in range(NOWN):
                nc.vector.scalar_tensor_tensor(
                    out=dump33[:],
                    in0=F_ps[:, m * NCLS:(m + 1) * NCLS],
                    scalar=1.0,
                    in1=O_own[:, m * NCLS:(m + 1) * NCLS],
                    op0=Alu.mult,
                    op1=Alu.mult,
                    accum_out=S_full[:, m:m + 1],
                )

            # Z excluding the diagonal, matching the in-matrix rounding exactly
            nc.vector.reduce_sum(
                Zrow[:], Zpart[:].rearrange("p (g m) -> p m g", m=NOWN),
                axis=mybir.AxisListType.X,
            )
            nc.scalar.activation(e_diag[:], rawdiag[:], Act.Exp, scale=INV_TAU)
            nc.vector.tensor_sub(Zexcl[:], Zrow[:], e_diag[:])
            nc.scalar.activation(lnZ[:], Zexcl[:], Act.Ln)

            # S excluding diagonal, scaled by 1/tau; positive count P
            nc.vector.tensor_sub(S_excl[:], S_full[:], rawdiag[:])
            nc.vector.tensor_scalar_add(P_pos[:], P_own[:], -1.0)
            nc.vector.tensor_scalar_max(P_safe[:], P_pos[:], 1.0)
            nc.vector.reciprocal(P_inv[:], P_safe[:])
            nc.vector.tensor_scalar(
                out=valid[:], in0=P_pos[:], scalar1=0.5, scalar2=None, op0=Alu.is_ge
            )
            # per_row = valid * (S*10*Pinv - lnZ)
            nc.vector.scalar_tensor_tensor(
                out=t_sp[:], in0=S_excl[:], scalar=INV_TAU, in1=P_inv[:],
                op0=Alu.mult, op1=Alu.mult,
            )
            nc.vector.tensor_sub(perrow[:], t_sp[:], lnZ[:])
            nc.vector.tensor_mul(perrow[:], perrow[:], valid[:])

            nc.vector.reduce_sum(
                loss_parts[:, 0:1], perrow[:],
                axis=mybir.AxisListType.X,
            )
            nc.vector.reduce_sum(
                loss_parts[:, 1:2], valid[:],
                axis=mybir.AxisListType.X,
            )
            sum_ps = epi_ps.tile([1, 2], f32, tag="sum")
            nc.tensor.matmul(sum_ps[:], ones_f[:], loss_parts[:], start=True, stop=True)
            nc.vector.tensor_copy(res_sb[:], sum_ps[:])
            nc.sync.dma_start(out_dram[:].rearrange("(a b) -> a b", a=1), res_sb[:])
            nc.sync.dma_start(dbg_zpart[:], Zpart[:])
            nc.sync.dma_start(dbg_rawdiag[:], rawdiag[:])
            nc.sync.dma_start(dbg_pown[:], P_own[:])
            nc.sync.dma_start(dbg_sfull[:], S_full[:])
            nc.sync.dma_start(dbg_cnt[:], cnt_bcast[:])
            nc.sync.dma_start(dbg_parts[:], loss_parts[:])
            xt_dbg = persist.tile([128, 256], f32)
            nc.vector.tensor_copy(xt_dbg[:], xT[:, 0:256])
            nc.sync.dma_start(dbg_xt[:], xt_dbg[:])

    if split_waits:
        tile_patch.split_multiwait(nc)
    return nc


def _get_nc(split_waits=True):
    global _NC
    if _NC is None:
        _NC = _build_nc(split_waits)
    return _NC


def _make_in_maps(x, lab):
    iota = np.ascontiguousarray(
        np.tile(np.arange(NCLS, dtype=np.float32), (128, 1))
    )
    eye = np.eye(128, dtype=np.float32)
    in_maps = []
    for c in range(N_CORES):
        lo, hi = c * ROWS_PER_CORE, (c + 1) * ROWS_PER_CORE
        perm = np.concatenate(
            [np.arange(lo, hi), np.arange(0, lo), np.arange(hi, N)]
        )
        xp = np.ascontiguousarray(x[perm])
        lp = np.ascontiguousarray(
            lab[perm].astype(np.float32).reshape(NCHUNK, 128).T
        )
        in_maps.append(
            {"xperm": xp, "labels_pc": lp, "iota33": iota, "identity": eye}
        )
    return in_maps


def _combine(results):
    parts = np.stack([np.asarray(results[c]["out"]) for c in range(N_CORES)])
    loss = -parts[:, 0].sum() / parts[:, 1].sum()
    return np.array(loss, dtype=np.float32)


def kernel(feature_embeds, label_ids):
    from concourse.bass_utils import run_bass_kernel_spmd

    x = np.asarray(feature_embeds, dtype=np.float32)
    lab = np.asarray(label_ids)
    nc = _get_nc()
    res = run_bass_kernel_spmd(nc, _make_in_maps(x, lab), list(range(N_CORES)))
    return _combine(res.results)


def kernel_profiled(feature_embeds, label_ids):
    """Same as kernel(), but with NTFF tracing; returns (loss, exec_time_ns)."""
    import profile_hook
    print("ntff hook installed:", profile_hook.install())
    from concourse.bass_utils import run_bass_kernel_spmd

    x = np.asarray(feature_embeds, dtype=np.float32)
    lab = np.asarray(label_ids)
    nc = _get_nc()
    res = run_bass_kernel_spmd(
        nc, _make_in_maps(x, lab), list(range(N_CORES)), trace=True
    )
    return _combine(res.results), res.exec_time_ns


# revision 9
# speedup vs baseline: 1.6380x; 1.2026x over previous
"""Supervised contrastive loss (nn_Batch_CL) on 8 Trainium2 NeuronCores.

Math (per the reference):
  x = l2_normalize(feature_embeds)            # [N, D]
  logits = (x @ x.T) / tau                    # tau = 0.1
  Z_i    = sum_{j != i} exp(logits[i, j])
  S_i    = sum_{j != i, l_j == l_i} logits[i, j]
  P_i    = |{j != i : l_j == l_i}|
  per_row_i = S_i / P_i - log Z_i   (if P_i > 0 else 0)
  loss = -sum(per_row) / n_valid

Distribution: rows sharded 8 ways (1024 rows/core). Each core receives the
full feature matrix with ITS OWN rows permuted to the front, so the diagonal
of its logits block lands at a statically-known position (cols m*128..+127 of
column-group 0 for row-chunk m) — no core-id branching; the SPMD program is
identical, only input data differs per core.

Per-core kernel strategy:
  - exp+row-sum fused in one ACT instruction per [128, 2048] PSUM block via
    activation(Exp, scale=10, accum_out=...): the Z reduction is free.
  - positive-pair sums via class aggregation: Msum = x_hat^T @ onehot(labels)
    accumulated on PE (borrowing a main-pool PSUM slot per group, drained to
    SBUF by a small DVE add), then F = x_hat_block @ Msum gives per-(row,
    class) sums; a one-hot mask + accum_out selects S_i. No NxN mask work.
  - exact diagonal terms extracted from the PSUM logits blocks with an
    identity-mask scalar_tensor_tensor + accum_out, so Z_i excludes e^{l_ii}
    bit-exactly and S_i excludes l_ii.
  - l2 normalization: rsqrt(s) = Exp(-0.5 * Ln(s)) on ACT — stays in the
    natural_log_exp table set used by the main exp (no table-set thrash).
  - x^T (contraction layout) built with batched bf16 DMA-xbar transposes
    (one [128, 8, 128] block-transpose instruction per 1024 columns).

Outputs per core: [sum of valid per_row over its 1024 rows, its n_valid].
Host epilogue: loss = -sum(parts) / sum(n_valid).
"""

import numpy as np

N = 8192
D = 128
N_CORES = 8
ROWS_PER_CORE = N // N_CORES          # 1024
NCHUNK = N // 128                     # 64 chunks of 128 rows
NGROUP = 4                            # column groups (ACT block = GW cols)
GW = N // NGROUP                      # 2048
HALF = GW // 2                        # build granularity (1024)
CH = HALF // 128                      # chunks per half-build (8)
NOWN = ROWS_PER_CORE // 128           # 8 own row-chunks
NCLS = 33
INV_TAU = 10.0
DEBUG_OUTPUTS = False

_NC = None


def _build_nc(split_waits=True):
    import concourse.bass as bass
    import concourse.mybir as mybir
    from concourse import tile
    from contextlib import ExitStack
    import tile_patch

    tile_patch.install()

    f32 = mybir.dt.float32
    bf16 = mybir.dt.bfloat16
    Alu = mybir.AluOpType
    Act = mybir.ActivationFunctionType
    X = mybir.AxisListType.X

    nc = bass.Bass()
    x_dram = nc.dram_tensor("xperm", [N, D], f32, kind="ExternalInput")
    lab_dram = nc.dram_tensor("labels_pc", [128, NCHUNK], f32, kind="ExternalInput")
    iota_dram = nc.dram_tensor("iota33", [128, NCLS], f32, kind="ExternalInput")
    eye_dram = nc.dram_tensor("identity", [128, 128], f32, kind="ExternalInput")
    out_dram = nc.dram_tensor("out", [2], f32, kind="ExternalOutput")
    if DEBUG_OUTPUTS:
        dbg = {
            name: nc.dram_tensor(name, shape, f32, kind="ExternalOutput")
            for name, shape in [
                ("dbg_zpart", [128, NGROUP * NOWN]),
                ("dbg_rawdiag", [128, NOWN]),
                ("dbg_pown", [128, NOWN]),
                ("dbg_sfull", [128, NOWN]),
                ("dbg_parts", [128, 2]),
            ]
        }

    with tile.TileContext(nc) as tc, ExitStack() as ctx:
        persist = ctx.enter_context(tc.tile_pool(name="persist", bufs=1))

        xT = persist.tile([128, N], bf16)                 # normalized, transposed
        O_bf = persist.tile([128, NCHUNK * NCLS], bf16)   # one-hot labels (PE operand)
        O_own = persist.tile([128, NOWN * NCLS], f32)     # one-hot, own chunks (DVE)
        cnt_bcast = persist.tile([128, NCLS], f32)
        Zpart = persist.tile([128, NGROUP * NOWN], f32)
        rawdiag = persist.tile([128, NOWN], f32)
        P_own = persist.tile([128, NOWN], f32)
        S_full = persist.tile([128, NOWN], f32)
        Msum_sb = persist.tile([128, NCLS], f32)          # accumulated class sums
        labels_sb = persist.tile([128, NCHUNK], f32)
        iota_sb = persist.tile([128, NCLS], f32)
        eye_sb = persist.tile([128, 128], f32)
        ones_f = persist.tile([128, 1], f32)
        ones_bf = persist.tile([128, 1], bf16)
        ones_row = persist.tile([1, 128], f32)
        cnt_row = persist.tile([1, NCLS], f32)
        Mt_sb = persist.tile([128, NCLS], bf16)
        dump128 = persist.tile([128, 128], f32)
        dump33 = persist.tile([128, NCLS], f32)
        msum_acc = persist.tile([128, NCLS], f32)
        e_dump = persist.tile([128, GW], f32)             # ACT out scratch (unread)
        res_sb = persist.tile([1, 2], f32)

        Zrow = persist.tile([128, NOWN], f32)
        e_diag = persist.tile([128, NOWN], f32)
        Zexcl = persist.tile([128, NOWN], f32)
        lnZ = persist.tile([128, NOWN], f32)
        S_excl = persist.tile([128, NOWN], f32)
        P_pos = persist.tile([128, NOWN], f32)
        P_safe = persist.tile([128, NOWN], f32)
        P_inv = persist.tile([128, NOWN], f32)
        valid = persist.tile([128, NOWN], f32)
        t_sp = persist.tile([128, NOWN], f32)
        perrow = persist.tile([128, NOWN], f32)
        loss_parts = persist.tile([128, 2], f32)

        # ---------------- prologue ----------------
        nc.sync.dma_start(labels_sb[:], lab_dram[:])
        nc.sync.dma_start(iota_sb[:], iota_dram[:])
        nc.sync.dma_start(eye_sb[:], eye_dram[:])
        nc.vector.memset(ones_f[:], 1.0)
        nc.vector.memset(ones_bf[:], 1.0)
        nc.vector.memset(ones_row[:], 1.0)
        nc.vector.memset(Msum_sb[:], 0.0)

        # one-hot build, single batched compare per tensor
        nc.vector.tensor_tensor(
            out=O_bf[:].rearrange("p (c k) -> p c k", k=NCLS),
            in0=iota_sb[:].rearrange("p (a k) -> p a k", a=1).to_broadcast(
                (128, NCHUNK, NCLS)),
            in1=labels_sb[:].to_broadcast((128, NCHUNK, NCLS)),
            op=Alu.is_equal,
        )
        nc.vector.tensor_tensor(
            out=O_own[:].rearrange("p (c k) -> p c k", k=NCLS),
            in0=iota_sb[:].rearrange("p (a k) -> p a k", a=1).to_broadcast(
                (128, NOWN, NCLS)),
            in1=labels_sb[:, 0:NOWN].to_broadcast((128, NOWN, NCLS)),
            op=Alu.is_equal,
        )

        with tc.tile_pool(name="pro_ps", bufs=1, space="PSUM") as pro_ps:
            cnt_ps = pro_ps.tile([1, NCLS], f32, tag="cnt")
            for c in range(NCHUNK):
                nc.tensor.matmul(
                    cnt_ps[:], ones_bf[:], O_bf[:, c * NCLS:(c + 1) * NCLS],
                    start=(c == 0), stop=(c == NCHUNK - 1),
                )
            nc.vector.tensor_copy(cnt_row[:], cnt_ps[:])
            cntb_ps = pro_ps.tile([128, NCLS], f32, tag="cntb")
            nc.tensor.matmul(cntb_ps[:], ones_row[:], cnt_row[:], start=True, stop=True)
            nc.vector.tensor_copy(cnt_bcast[:], cntb_ps[:])

        for m in range(NOWN):
            nc.vector.scalar_tensor_tensor(
                out=dump33[:],
                in0=O_own[:, m * NCLS:(m + 1) * NCLS],
                scalar=1.0,
                in1=cnt_bcast[:],
                op0=Alu.mult,
                op1=Alu.mult,
                accum_out=P_own[:, m:m + 1],
            )

        # ---------------- main: build + compute, group by group ----------------
        with (
            tc.tile_pool(name="main_ps", bufs=2, space="PSUM") as main_ps,
            tc.tile_pool(name="build", bufs=2) as build_pool,
        ):
            for g in range(NGROUP):
                # --- build group g of xT: two half-builds of 1024 cols ---
                xh_halves = []
                for h in range(2):
                    base = g * GW + h * HALF          # column offset
                    xs = build_pool.tile([128, HALF], f32, tag=f"xs{h}")
                    nc.sync.dma_start(
                        xs[:].rearrange("p (c d) -> p c d", d=128),
                        x_dram[base:base + HALF, :].rearrange(
                            "(c p) d -> p c d", p=128),
                    )
                    sq = build_pool.tile([128, HALF], f32, tag=f"sq{h}")
                    nc.vector.tensor_mul(sq[:], xs[:], xs[:])
                    ssq = build_pool.tile([128, CH], f32, tag=f"ssq{h}")
                    nc.vector.reduce_sum(
                        ssq[:], sq[:].rearrange("p (c d) -> p c d", d=128), axis=X)
                    lns = build_pool.tile([128, CH], f32, tag=f"lns{h}")
                    nc.scalar.activation(lns[:], ssq[:], Act.Ln)
                    rinv = build_pool.tile([128, CH], f32, tag=f"rinv{h}")
                    nc.scalar.activation(rinv[:], lns[:], Act.Exp, scale=-0.5)
                    xh = build_pool.tile([128, HALF], bf16, tag=f"xh{h}")
                    nc.vector.scalar_tensor_tensor(
                        out=xh[:].rearrange("p (c r) -> p c r", r=128),
                        in0=xs[:].rearrange("p (c r) -> p c r", r=128),
                        scalar=1.0,
                        in1=rinv[:].to_broadcast((128, CH, 128)),
                        op0=Alu.mult,
                        op1=Alu.mult,
                    )
                    nc.sync.dma_start_transpose(
                        xT[:, base:base + HALF].rearrange("p (c r) -> p c r", r=128),
                        xh[:],
                    )
                    xh_halves.append(xh)

                # --- class-sum accumulation: borrow a main-pool slot ---
                mps = main_ps.tile([128, GW], f32, tag="e")
                for h in range(2):
                    for i in range(CH):
                        c = g * (GW // 128) + h * CH + i
                        nc.tensor.matmul(
                            mps[:, 0:NCLS],
                            xh_halves[h][:, i * 128:(i + 1) * 128],
                            O_bf[:, c * NCLS:(c + 1) * NCLS],
                            start=(h == 0 and i == 0),
                            stop=(h == 1 and i == CH - 1),
                        )
                nc.vector.scalar_tensor_tensor(
                    out=msum_acc[:], in0=mps[:, 0:NCLS], scalar=1.0, in1=Msum_sb[:],
                    op0=Alu.mult, op1=Alu.add,
                )
                nc.vector.tensor_copy(Msum_sb[:], msum_acc[:])

                # --- logits + exp + rowsum for all 8 own row-chunks ---
                for m in range(NOWN):
                    ps = main_ps.tile([128, GW], f32, tag="e")
                    lhsT = xT[:, m * 128:(m + 1) * 128]
                    for k in range(GW // 512):
                        nc.tensor.matmul(
                            ps[:, k * 512:(k + 1) * 512],
                            lhsT,
                            xT[:, g * GW + k * 512: g * GW + (k + 1) * 512],
                            start=True, stop=True,
                        )
                    if g == 0:
                        nc.vector.scalar_tensor_tensor(
                            out=dump128[:],
                            in0=ps[:, m * 128:(m + 1) * 128],
                            scalar=1.0,
                            in1=eye_sb[:],
                            op0=Alu.mult,
                            op1=Alu.mult,
                            accum_out=rawdiag[:, m:m + 1],
                        )
                    nc.scalar.activation(
                        e_dump[:], ps[:], Act.Exp, scale=INV_TAU,
                        accum_out=Zpart[:, g * NOWN + m: g * NOWN + m + 1],
                    )

        nc.vector.tensor_copy(Mt_sb[:], Msum_sb[:])

        # ---------------- epilogue ----------------
        with tc.tile_pool(name="epi_ps", bufs=1, space="PSUM") as epi_ps:
            F_ps = epi_ps.tile([128, NOWN * NCLS], f32, tag="F")
            for m in range(NOWN):
                nc.tensor.matmul(
                    F_ps[:, m * NCLS:(m + 1) * NCLS],
                    xT[:, m * 128:(m + 1) * 128],
                    Mt_sb[:],
                    start=True, stop=True,
                )
            for m in range(NOWN):
                nc.vector.scalar_tensor_tensor(
                    out=dump33[:],
                    in0=F_ps[:, m * NCLS:(m + 1) * NCLS],
                    scalar=1.0,
                    in1=O_own[:, m * NCLS:(m + 1) * NCLS],
                    op0=Alu.mult,
                    op1=Alu.mult,
                    accum_out=S_full[:, m:m + 1],
                )

            nc.vector.reduce_sum(
                Zrow[:], Zpart[:].rearrange("p (g m) -> p m g", m=NOWN), axis=X)
            nc.scalar.activation(e_diag[:], rawdiag[:], Act.Exp, scale=INV_TAU)
            nc.vector.tensor_sub(Zexcl[:], Zrow[:], e_diag[:])
            nc.scalar.activation(lnZ[:], Zexcl[:], Act.Ln)

            nc.vector.tensor_sub(S_excl[:], S_full[:], rawdiag[:])
            nc.vector.tensor_scalar_add(P_pos[:], P_own[:], -1.0)
            nc.vector.tensor_scalar_max(P_safe[:], P_pos[:], 1.0)
            nc.vector.reciprocal(P_inv[:], P_safe[:])
            nc.vector.tensor_scalar_min(valid[:], P_pos[:], 1.0)  # P>=0 integer
            nc.vector.scalar_tensor_tensor(
                out=t_sp[:], in0=S_excl[:], scalar=INV_TAU, in1=P_inv[:],
                op0=Alu.mult, op1=Alu.mult,
            )
            nc.vector.tensor_sub(perrow[:], t_sp[:], lnZ[:])
            nc.vector.tensor_mul(perrow[:], perrow[:], valid[:])

            nc.vector.reduce_sum(loss_parts[:, 0:1], perrow[:], axis=X)
            nc.vector.reduce_sum(loss_parts[:, 1:2], valid[:], axis=X)
            sum_ps = epi_ps.tile([1, 2], f32, tag="sum")
            nc.tensor.matmul(sum_ps[:], ones_f[:], loss_parts[:], start=True, stop=True)
            nc.vector.tensor_copy(res_sb[:], sum_ps[:])
            nc.sync.dma_start(out_dram[:].rearrange("(a b) -> a b", a=1), res_sb[:])
            if DEBUG_OUTPUTS:
                nc.sync.dma_start(dbg["dbg_zpart"][:], Zpart[:])
                nc.sync.dma_start(dbg["dbg_rawdiag"][:], rawdiag[:])
                nc.sync.dma_start(dbg["dbg_pown"][:], P_own[:])
                nc.sync.dma_start(dbg["dbg_sfull"][:], S_full[:])
                nc.sync.dma_start(dbg["dbg_parts"][:], loss_parts[:])

    if split_waits:
        tile_patch.split_multiwait(nc)
    return nc


def _get_nc(split_waits=True):
    global _NC
    if _NC is None:
        _NC = _build_nc(split_waits)
    return _NC


def _make_in_maps(x, lab):
    iota = np.ascontiguousarray(
        np.tile(np.arange(NCLS, dtype=np.float32), (128, 1))
    )
    eye = np.eye(128, dtype=np.float32)
    in_maps = []
    for c in range(N_CORES):
        lo, hi = c * ROWS_PER_CORE, (c + 1) * ROWS_PER_CORE
        perm = np.concatenate(
            [np.arange(lo, hi), np.arange(0, lo), np.arange(hi, N)]
        )
        xp = np.ascontiguousarray(x[perm])
        lp = np.ascontiguousarray(
            lab[perm].astype(np.float32).reshape(NCHUNK, 128).T
        )
        in_maps.append(
            {"xperm": xp, "labels_pc": lp, "iota33": iota, "identity": eye}
        )
    return in_maps


def _combine(results):
    parts = np.stack([np.asarray(results[c]["out"]) for c in range(N_CORES)])
    loss = -parts[:, 0].sum() / parts[:, 1].sum()
    return np.array(loss, dtype=np.float32)


def kernel(feature_embeds, label_ids):
    from concourse.bass_utils import run_bass_kernel_spmd

    x = np.asarray(feature_embeds, dtype=np.float32)
    lab = np.asarray(label_ids)
    nc = _get_nc()
    res = run_bass_kernel_spmd(nc, _make_in_maps(x, lab), list(range(N_CORES)))
    return _combine(res.results)


def kernel_profiled(feature_embeds, label_ids):
    """Same as kernel(), but with NTFF tracing; returns (loss, exec_time_ns)."""
    import profile_hook
    print("ntff hook installed:", profile_hook.install())
    from concourse.bass_utils import run_bass_kernel_spmd

    x = np.asarray(feature_embeds, dtype=np.float32)
    lab = np.asarray(label_ids)
    nc = _get_nc()
    res = run_bass_kernel_spmd(
        nc, _make_in_maps(x, lab), list(range(N_CORES)), trace=True
    )
    return _combine(res.results), res.exec_time_ns


# revision 11
# speedup vs baseline: 1.6473x; 1.0057x over previous
"""Supervised contrastive loss (nn_Batch_CL) on 8 Trainium2 NeuronCores.

Math (per the reference):
  x = l2_normalize(feature_embeds)            # [N, D]
  logits = (x @ x.T) / tau                    # tau = 0.1
  Z_i    = sum_{j != i} exp(logits[i, j])
  S_i    = sum_{j != i, l_j == l_i} logits[i, j]
  P_i    = |{j != i : l_j == l_i}|
  per_row_i = S_i / P_i - log Z_i   (if P_i > 0 else 0)
  loss = -sum(per_row) / n_valid

Distribution: rows sharded 8 ways (1024 rows/core). Each core receives the
full feature matrix with ITS OWN rows permuted to the front, so the diagonal
of its logits block lands at a statically-known position (cols m*128..+127 of
column-group 0 for row-chunk m) — no core-id branching; the SPMD program is
identical, only input data differs per core.

Per-core kernel strategy:
  - exp+row-sum fused in one ACT instruction per [128, 2048] PSUM block via
    activation(Exp, scale=10, accum_out=...): the Z reduction is free.
  - positive-pair sums via class aggregation: Msum = x_hat^T @ onehot(labels)
    accumulated on PE (borrowing a main-pool PSUM slot per group, drained to
    SBUF by a small DVE add), then F = x_hat_block @ Msum gives per-(row,
    class) sums; a one-hot mask + accum_out selects S_i. No NxN mask work.
  - exact diagonal terms extracted from the PSUM logits blocks with an
    identity-mask scalar_tensor_tensor + accum_out, so Z_i excludes e^{l_ii}
    bit-exactly and S_i excludes l_ii.
  - l2 normalization: rsqrt(s) = Exp(-0.5 * Ln(s)) on ACT — stays in the
    natural_log_exp table set used by the main exp (no table-set thrash).
  - x^T (contraction layout) built with batched bf16 DMA-xbar transposes
    (one [128, 8, 128] block-transpose instruction per 1024 columns).

Outputs per core: [sum of valid per_row over its 1024 rows, its n_valid].
Host epilogue: loss = -sum(parts) / sum(n_valid).
"""

import numpy as np

N = 8192
D = 128
N_CORES = 8
ROWS_PER_CORE = N // N_CORES          # 1024
NCHUNK = N // 128                     # 64 chunks of 128 rows
GROUPS = [1024, 2048, 2048, 2048, 1024]   # column group widths
NGROUP = len(GROUPS)
GW = 2048                             # max group width (psum tile size)
HALF = 1024                           # build granularity
CH = HALF // 128                      # chunks per half-build (8)
NOWN = ROWS_PER_CORE // 128           # 8 own row-chunks
NCLS = 33
INV_TAU = 10.0
DEBUG_OUTPUTS = False

_NC = None


def _build_nc(split_waits=True):
    import concourse.bass as bass
    import concourse.mybir as mybir
    from concourse import tile
    from contextlib import ExitStack
    import tile_patch

    tile_patch.install()

    f32 = mybir.dt.float32
    bf16 = mybir.dt.bfloat16
    Alu = mybir.AluOpType
    Act = mybir.ActivationFunctionType
    X = mybir.AxisListType.X

    nc = bass.Bass()
    x_dram = nc.dram_tensor("xperm", [N, D], f32, kind="ExternalInput")
    lab_dram = nc.dram_tensor("labels_pc", [128, NCHUNK], f32, kind="ExternalInput")
    iota_dram = nc.dram_tensor("iota33", [128, NCLS], f32, kind="ExternalInput")
    eye_dram = nc.dram_tensor("identity", [128, 128], f32, kind="ExternalInput")
    out_dram = nc.dram_tensor("out", [2], f32, kind="ExternalOutput")
    if DEBUG_OUTPUTS:
        dbg = {
            name: nc.dram_tensor(name, shape, f32, kind="ExternalOutput")
            for name, shape in [
                ("dbg_zpart", [128, NGROUP * NOWN]),
                ("dbg_rawdiag", [128, NOWN]),
                ("dbg_pown", [128, NOWN]),
                ("dbg_sfull", [128, NOWN]),
                ("dbg_parts", [128, 2]),
            ]
        }

    with tile.TileContext(nc) as tc, ExitStack() as ctx:
        persist = ctx.enter_context(tc.tile_pool(name="persist", bufs=1))

        xT = persist.tile([128, N], bf16)                 # normalized, transposed
        O_bf = persist.tile([128, NCHUNK * NCLS], bf16)   # one-hot labels (PE operand)
        O_own = persist.tile([128, NOWN * NCLS], f32)     # one-hot, own chunks (DVE)
        cnt_bcast = persist.tile([128, NCLS], f32)
        Zpart = persist.tile([128, NGROUP * NOWN], f32)
        rawdiag = persist.tile([128, NOWN], f32)
        P_own = persist.tile([128, NOWN], f32)
        S_full = persist.tile([128, NOWN], f32)
        Msum_sb = persist.tile([NCLS, 128], f32)          # class sums [cls, d]
        labels_sb = persist.tile([128, NCHUNK], f32)
        iota_sb = persist.tile([128, NCLS], f32)
        eye_sb = persist.tile([128, 128], f32)
        ones_f = persist.tile([128, 1], f32)
        ones_bf = persist.tile([128, 1], bf16)
        ones_row = persist.tile([1, 128], f32)
        cnt_row = persist.tile([1, NCLS], f32)
        Mt_sb = persist.tile([128, NCLS], bf16)
        dump128 = persist.tile([128, 128], f32)
        dump33 = persist.tile([128, NCLS], f32)
        msum_acc = persist.tile([NCLS, 128], f32)
        e_dump = persist.tile([128, GW], f32)             # ACT out scratch (unread)
        res_sb = persist.tile([1, 2], f32)

        Zrow = persist.tile([128, NOWN], f32)
        e_diag = persist.tile([128, NOWN], f32)
        Zexcl = persist.tile([128, NOWN], f32)
        lnZ = persist.tile([128, NOWN], f32)
        S_excl = persist.tile([128, NOWN], f32)
        P_pos = persist.tile([128, NOWN], f32)
        P_safe = persist.tile([128, NOWN], f32)
        P_inv = persist.tile([128, NOWN], f32)
        valid = persist.tile([128, NOWN], f32)
        t_sp = persist.tile([128, NOWN], f32)
        perrow = persist.tile([128, NOWN], f32)
        loss_parts = persist.tile([128, 2], f32)

        # ---------------- prologue ----------------
        nc.sync.dma_start(labels_sb[:], lab_dram[:])
        nc.sync.dma_start(iota_sb[:], iota_dram[:])
        nc.sync.dma_start(eye_sb[:], eye_dram[:])
        nc.vector.memset(ones_f[:], 1.0)
        nc.vector.memset(ones_bf[:], 1.0)
        nc.vector.memset(ones_row[:], 1.0)
        nc.vector.memset(Msum_sb[:], 0.0)

        # one-hot build, single batched compare per tensor
        nc.vector.tensor_tensor(
            out=O_bf[:].rearrange("p (c k) -> p c k", k=NCLS),
            in0=iota_sb[:].rearrange("p (a k) -> p a k", a=1).to_broadcast(
                (128, NCHUNK, NCLS)),
            in1=labels_sb[:].to_broadcast((128, NCHUNK, NCLS)),
            op=Alu.is_equal,
        )
        nc.vector.tensor_tensor(
            out=O_own[:].rearrange("p (c k) -> p c k", k=NCLS),
            in0=iota_sb[:].rearrange("p (a k) -> p a k", a=1).to_broadcast(
                (128, NOWN, NCLS)),
            in1=labels_sb[:, 0:NOWN].to_broadcast((128, NOWN, NCLS)),
            op=Alu.is_equal,
        )

        cnt_part = persist.tile([128, NCLS], f32)
        nc.vector.reduce_sum(
            cnt_part[:], O_bf[:].rearrange("p (c k) -> p k c", k=NCLS), axis=X)
        with tc.tile_pool(name="pro_ps", bufs=1, space="PSUM") as pro_ps:
            cnt_ps = pro_ps.tile([1, NCLS], f32, tag="cnt")
            nc.tensor.matmul(cnt_ps[:], ones_f[:], cnt_part[:], start=True, stop=True)
            nc.vector.tensor_copy(cnt_row[:], cnt_ps[:])
            cntb_ps = pro_ps.tile([128, NCLS], f32, tag="cntb")
            nc.tensor.matmul(cntb_ps[:], ones_row[:], cnt_row[:], start=True, stop=True)
            nc.vector.tensor_copy(cnt_bcast[:], cntb_ps[:])

        for m in range(NOWN):
            nc.vector.scalar_tensor_tensor(
                out=dump33[:],
                in0=O_own[:, m * NCLS:(m + 1) * NCLS],
                scalar=1.0,
                in1=cnt_bcast[:],
                op0=Alu.mult,
                op1=Alu.mult,
                accum_out=P_own[:, m:m + 1],
            )

        # ---------------- main: build + compute, group by group ----------------
        with (
            tc.tile_pool(name="main_ps", bufs=2, space="PSUM") as main_ps,
            tc.tile_pool(name="build", bufs=2) as build_pool,
        ):
            gstart = 0
            for g, gw in enumerate(GROUPS):
                nhalf = gw // HALF
                # --- build group g of xT: half-builds of 1024 cols ---
                xh_halves = []
                for h in range(nhalf):
                    base = gstart + h * HALF          # column offset
                    xs = build_pool.tile([128, HALF], f32, tag=f"xs{h}")
                    nc.sync.dma_start(
                        xs[:].rearrange("p (c d) -> p c d", d=128),
                        x_dram[base:base + HALF, :].rearrange(
                            "(c p) d -> p c d", p=128),
                    )
                    sq = build_pool.tile([128, HALF], f32, tag=f"sq{h}")
                    nc.vector.tensor_mul(sq[:], xs[:], xs[:])
                    ssq = build_pool.tile([128, CH], f32, tag=f"ssq{h}")
                    nc.vector.reduce_sum(
                        ssq[:], sq[:].rearrange("p (c d) -> p c d", d=128), axis=X)
                    lns = build_pool.tile([128, CH], f32, tag=f"lns{h}")
                    nc.scalar.activation(lns[:], ssq[:], Act.Ln)
                    rinv = build_pool.tile([128, CH], f32, tag=f"rinv{h}")
                    nc.scalar.activation(rinv[:], lns[:], Act.Exp, scale=-0.5)
                    xh = build_pool.tile([128, HALF], bf16, tag=f"xh{h}")
                    nc.vector.scalar_tensor_tensor(
                        out=xh[:].rearrange("p (c r) -> p c r", r=128),
                        in0=xs[:].rearrange("p (c r) -> p c r", r=128),
                        scalar=1.0,
                        in1=rinv[:].to_broadcast((128, CH, 128)),
                        op0=Alu.mult,
                        op1=Alu.mult,
                    )
                    nc.sync.dma_start_transpose(
                        xT[:, base:base + HALF].rearrange("p (c r) -> p c r", r=128),
                        xh[:],
                    )
                    xh_halves.append(xh)

                # --- logits + exp + rowsum for all 8 own row-chunks ---
                for m in range(NOWN):
                    ps = main_ps.tile([128, GW], f32, tag="e")
                    lhsT = xT[:, m * 128:(m + 1) * 128]
                    for k in range(gw // 512):
                        nc.tensor.matmul(
                            ps[:, k * 512:(k + 1) * 512],
                            lhsT,
                            xT[:, gstart + k * 512: gstart + (k + 1) * 512],
                            start=True, stop=True,
                        )
                    if g == 0:
                        nc.vector.scalar_tensor_tensor(
                            out=dump128[:],
                            in0=ps[:, m * 128:(m + 1) * 128],
                            scalar=1.0,
                            in1=eye_sb[:],
                            op0=Alu.mult,
                            op1=Alu.mult,
                            accum_out=rawdiag[:, m:m + 1],
                        )
                    nc.scalar.activation(
                        e_dump[:, 0:gw], ps[:, 0:gw], Act.Exp, scale=INV_TAU,
                        accum_out=Zpart[:, g * NOWN + m: g * NOWN + m + 1],
                    )

                # --- class-sum accumulation (off the ACT feed path):
                #     lhsT = one-hot chunk (33-col LDWEIGHTS), out = [33, 128]
                mps = main_ps.tile([128, GW], f32, tag="e")
                for h in range(nhalf):
                    for i in range(CH):
                        c = gstart // 128 + h * CH + i
                        nc.tensor.matmul(
                            mps[0:NCLS, 0:128],
                            O_bf[:, c * NCLS:(c + 1) * NCLS],
                            xh_halves[h][:, i * 128:(i + 1) * 128],
                            start=(h == 0 and i == 0),
                            stop=(h == nhalf - 1 and i == CH - 1),
                        )
                nc.vector.scalar_tensor_tensor(
                    out=msum_acc[:], in0=mps[0:NCLS, 0:128], scalar=1.0,
                    in1=Msum_sb[:], op0=Alu.mult, op1=Alu.add,
                )
                nc.vector.tensor_copy(Msum_sb[:], msum_acc[:])
                gstart += gw

        # ---------------- epilogue ----------------
        with tc.tile_pool(name="epi_ps", bufs=1, space="PSUM") as epi_ps:
            mt_ps = epi_ps.tile([128, NCLS], f32, tag="mt")
            nc.tensor.transpose(mt_ps[:], Msum_sb[:], eye_sb[0:NCLS, 0:NCLS])
            nc.vector.tensor_copy(Mt_sb[:], mt_ps[:])
            F_ps = epi_ps.tile([128, NOWN * NCLS], f32, tag="F")
            for m in range(NOWN):
                nc.tensor.matmul(
                    F_ps[:, m * NCLS:(m + 1) * NCLS],
                    xT[:, m * 128:(m + 1) * 128],
                    Mt_sb[:],
                    start=True, stop=True,
                )
            for m in range(NOWN):
                nc.vector.scalar_tensor_tensor(
                    out=dump33[:],
                    in0=F_ps[:, m * NCLS:(m + 1) * NCLS],
                    scalar=1.0,
                    in1=O_own[:, m * NCLS:(m + 1) * NCLS],
                    op0=Alu.mult,
                    op1=Alu.mult,
                    accum_out=S_full[:, m:m + 1],
                )

            nc.vector.reduce_sum(
                Zrow[:], Zpart[:].rearrange("p (g m) -> p m g", m=NOWN), axis=X)
            nc.scalar.activation(e_diag[:], rawdiag[:], Act.Exp, scale=INV_TAU)
            nc.vector.tensor_sub(Zexcl[:], Zrow[:], e_diag[:])
            nc.scalar.activation(lnZ[:], Zexcl[:], Act.Ln)

            nc.vector.tensor_sub(S_excl[:], S_full[:], rawdiag[:])
            nc.vector.tensor_scalar_add(P_pos[:], P_own[:], -1.0)
            nc.vector.tensor_scalar_max(P_safe[:], P_pos[:], 1.0)
            nc.vector.reciprocal(P_inv[:], P_safe[:])
            nc.vector.tensor_scalar_min(valid[:], P_pos[:], 1.0)  # P>=0 integer
            nc.vector.scalar_tensor_tensor(
                out=t_sp[:], in0=S_excl[:], scalar=INV_TAU, in1=P_inv[:],
                op0=Alu.mult, op1=Alu.mult,
            )
            nc.vector.tensor_sub(perrow[:], t_sp[:], lnZ[:])
            nc.vector.tensor_mul(perrow[:], perrow[:], valid[:])

            nc.vector.reduce_sum(loss_parts[:, 0:1], perrow[:], axis=X)
            nc.vector.reduce_sum(loss_parts[:, 1:2], valid[:], axis=X)
            sum_ps = epi_ps.tile([1, 2], f32, tag="sum")
            nc.tensor.matmul(sum_ps[:], ones_f[:], loss_parts[:], start=True, stop=True)
            nc.vector.tensor_copy(res_sb[:], sum_ps[:])
            nc.sync.dma_start(out_dram[:].rearrange("(a b) -> a b", a=1), res_sb[:])
            if DEBUG_OUTPUTS:
                nc.sync.dma_start(dbg["dbg_zpart"][:], Zpart[:])
                nc.sync.dma_start(dbg["dbg_rawdiag"][:], rawdiag[:])
                nc.sync.dma_start(dbg["dbg_pown"][:], P_own[:])
                nc.sync.dma_start(dbg["dbg_sfull"][:], S_full[:])
                nc.sync.dma_start(dbg["dbg_parts"][:], loss_parts[:])

    if split_waits:
        tile_patch.split_multiwait(nc)
    return nc


def _get_nc(split_waits=True):
    global _NC
    if _NC is None:
        _NC = _build_nc(split_waits)
    return _NC


def _make_in_maps(x, lab):
    iota = np.ascontiguousarray(
        np.tile(np.arange(NCLS, dtype=np.float32), (128, 1))
    )
    eye = np.eye(128, dtype=np.float32)
    in_maps = []
    for c in range(N_CORES):
        lo, hi = c * ROWS_PER_CORE, (c + 1) * ROWS_PER_CORE
        perm = np.concatenate(
            [np.arange(lo, hi), np.arange(0, lo), np.arange(hi, N)]
        )
        xp = np.ascontiguousarray(x[perm])
        lp = np.ascontiguousarray(
            lab[perm].astype(np.float32).reshape(NCHUNK, 128).T
        )
        in_maps.append(
            {"xperm": xp, "labels_pc": lp, "iota33": iota, "identity": eye}
        )
    return in_maps


def _combine(results):
    parts = np.stack([np.asarray(results[c]["out"]) for c in range(N_CORES)])
    loss = -parts[:, 0].sum() / parts[:, 1].sum()
    return np.array(loss, dtype=np.float32)


def kernel(feature_embeds, label_ids):
    from concourse.bass_utils import run_bass_kernel_spmd

    x = np.asarray(feature_embeds, dtype=np.float32)
    lab = np.asarray(label_ids)
    nc = _get_nc()
    res = run_bass_kernel_spmd(nc, _make_in_maps(x, lab), list(range(N_CORES)))
    return _combine(res.results)


def kernel_profiled(feature_embeds, label_ids):
    """Same as kernel(), but with NTFF tracing; returns (loss, exec_time_ns)."""
    import profile_hook
    print("ntff hook installed:", profile_hook.install())
    from concourse.bass_utils import run_bass_kernel_spmd

    x = np.asarray(feature_embeds, dtype=np.float32)
    lab = np.asarray(label_ids)
    nc = _get_nc()
    res = run_bass_kernel_spmd(
        nc, _make_in_maps(x, lab), list(range(N_CORES)), trace=True
    )
    return _combine(res.results), res.exec_time_ns
